# revision 62
# baseline (speedup 1.0000x reference)
"""Trainium2 Bass kernel for nn_Attention_4741643894804 (sparse_attention).

Reference computation (T=2048, B=32, N=1024, N2=512):
    m  = mean_t hyp[t, b, :]                       # [B, N]
    x  = hyp.transpose(1, 0, 2)                    # [B, T, N]
    hq = tanh(x @ W_w.T + W_b)                     # [B, T, N2]
    hm = tanh(m @ Wm_w.T + Wm_b)                   # [B, N2]
    s  = (hq * hm[:, None, :]) @ Wh_w.T + Wh_b     # [B, T, 1]
    a  = softmax(s, axis=1)
    c  = sum_t a * x                               # [B, N]

Strategy: pure data parallel over B (4 batches per core, 8 cores, no
collectives).  Per batch:
  - hq^T = tanh(W_w @ x + W_b) as [k2, t] tiles on TensorE in fp8
    DoubleRow (e4m3; W_w prescaled x32 on the host to dodge subnormals,
    undone by the tanh activation's scale); tanh output stored fp8.
  - mean m_raw = sum_t x from bf16 x via in-place pairwise add trees on
    DVE (split per n-half so each half starts as soon as its DMA lands).
  - hm row s64[1, k2] via fp8 DoubleRow with the tiny mean as lhsT (m/2
    in fp8, Wm prescaled x32; no weight-load cost), wm_b folded in via a
    ones matmul against a host-prescaled bias row; v = 64*tanh(.)*Wh_w
    on ACT+DVE, transposed to fp8 columns with 4 tiny K=1 matmuls.
  - score rows s64[1, t] = v8 . hq8 via fp8 DoubleRow (x64 undone by the
    exp scale); e = exp(score) unnormalized (softmax is shift-invariant
    and scores are tiny for this data).
  - exp rows land on partitions 32/64 of a zeroed [65, 2, TCH] tile
    (ones row at 0); one K=65 matmul per 128-slice against a constant
    [[-64,-64],[64,0],[0,64]]-padded rhs transposes AND applies 64*(e-1)
    in one shot -> delta columns.
  - weighted sum c_u = sum_t 64*(e_t-1) x_t on TensorE against a [t, n]
    fp8 copy of x (DoubleRow); sum(e) = sum(dcol)/64 + T via 8 tiny
    accumulating matmuls, so no f32 reductions are needed.
  - c = rs*(cu/64 + m_raw), rs = 1/sum(e): the delta term is ~1% of c so
    it tolerates fp8; m_raw supplies the dominant part at bf16 precision.

Scheduling: batch b's score/softmax/weighted-sum tail is emitted
interleaved between batch b+1's hq matmul groups, so the in-order PE
never stalls on the ACT exp chain; hm(b) is emitted at the end of
HQT(b), a full batch ahead of its first use.
"""

import os
import sys

import numpy as np

if "/opt/trn_rl_repo" not in sys.path:
    sys.path.insert(0, "/opt/trn_rl_repo")

import ml_dtypes

T, B, N, N2 = 2048, 32, 1024, 512
NCORES = 8
BLOC = B // NCORES  # 4 batches per core
NC8 = N // 128      # 8 n-chunks of 128
K2C = N2 // 128     # 4 k2-chunks of 128
TCH = 512           # t tile (one PSUM bank of f32)
TC = T // TCH       # 4 t-chunks

# wsum matmul pj -> dcol column index (transposes write 2 adjacent cols)
IDX = [0, 2, 1, 3, 4, 6, 5, 7]

_CACHE = {}
LAST_EXEC_NS = None


def _build(loop_n=None, stage=3):
    """Build the Bass graph.  loop_n wraps the whole computation in an
    on-device For_i loop (benchmarking only).  stage ablates work for HW
    attribution: 0=DMA only, 1=+hq matmuls, 2=+tanh/score/exp/wsum,
    3=full (default)."""
    import contextlib

    import concourse.bass as bass
    import concourse.mybir as mybir
    import concourse.tile as tile
    from concourse import bacc
    from concourse.bass import ts

    fp32 = mybir.dt.float32
    bf16 = mybir.dt.bfloat16
    fp8 = mybir.dt.float8e4
    AF = mybir.ActivationFunctionType
    ALU = mybir.AluOpType
    AX = mybir.AxisListType
    DR = mybir.MatmulPerfMode.DoubleRow

    # ablation flags per stage
    ALL = dict(x=1, xf8=1, xtn=1, hq=1, tanh=1, sc=1, tr=1, wsum=1, fin=1,
               hm=1, tree=1, end=1)
    CFG = {
        0: dict(ALL, hq=0, tanh=0, sc=0, tr=0, wsum=0, fin=0, hm=0, tree=0,
                end=0),
        1: dict(ALL, tanh=0, sc=0, tr=0, wsum=0, fin=0, hm=0, tree=0, end=0),
        2: dict(ALL, hm=0, tree=0, end=0),
        3: ALL,
        4: dict(ALL, sc=0, tr=0, wsum=0, fin=0, hm=0, tree=0, end=0),
        5: dict(ALL, tr=0, wsum=0, fin=0, hm=0, tree=0, end=0),
        6: dict(ALL, wsum=0, fin=0, hm=0, tree=0, end=0),
        7: dict(ALL, fin=0, hm=0, tree=0, end=0),
        11: dict(ALL, x=0, xtn=0, tanh=0, sc=0, tr=0, wsum=0, fin=0, hm=0,
                 tree=0, end=0),
        12: dict(ALL, x=0, xtn=0, sc=0, tr=0, wsum=0, fin=0, hm=0, tree=0,
                 end=0),
        20: dict(ALL, x=0, xf8=0, xtn=0),
        21: dict(ALL, xf8=0, xtn=0),
    }
    cfg = CFG[stage]

    nc = bacc.Bacc("TRN2", target_bir_lowering=False, debug=False,
                   num_devices=NCORES)

    x_ext = nc.declare_dram_parameter("x", [BLOC, 2, 128, NC8 // 2, T], bf16,
                                      isOutput=False)
    xf8_ext = nc.declare_dram_parameter("xf8", [BLOC, TC, 128, NC8, TCH],
                                        fp8, isOutput=False)
    xtn_ext = nc.declare_dram_parameter(
        "xtn", [BLOC, 128, T // 256, 2, N], fp8, isOutput=False)
    wtf8_ext = nc.declare_dram_parameter("wtf8", [128, NC8, K2C, 128], fp8,
                                         isOutput=False)
    wmt8_ext = nc.declare_dram_parameter("wmt8", [128, NC8, N2], fp8,
                                         isOutput=False)
    wb_ext = nc.declare_dram_parameter("wb", [128, K2C], fp32, isOutput=False)
    wmbr_ext = nc.declare_dram_parameter("wmbr", [1, N2], bf16,
                                         isOutput=False)
    whw64_ext = nc.declare_dram_parameter("whw64", [1, N2], bf16,
                                          isOutput=False)
    g65_ext = nc.declare_dram_parameter("g65", [65, 2], bf16, isOutput=False)
    i97_ext = nc.declare_dram_parameter("i97", [97, 4], bf16, isOutput=False)
    out_ext = nc.declare_dram_parameter("out", [128, BLOC * NC8], fp32,
                                        isOutput=True)

    with tile.TileContext(nc) as tc:
        with (
            tc.tile_pool(name="consts", bufs=1) as consts,
            tc.tile_pool(name="xp", bufs=3) as xp,
            tc.tile_pool(name="hqp", bufs=2) as hqp,
            tc.tile_pool(name="scr", bufs=2) as scrp,
            tc.tile_pool(name="abp", bufs=2) as abp,
            tc.tile_pool(name="smalls", bufs=2) as smalls,
            tc.tile_pool(name="hq_ps", bufs=4, space="PSUM") as hq_psp,
            tc.tile_pool(name="sm_ps", bufs=1, space="PSUM") as sm_psp,
            tc.tile_pool(name="d_ps", bufs=1, space="PSUM") as d_psp,
        ):
            # wtf8 first: it gates the very first hq matmul
            wt_sb = consts.tile([128, NC8, K2C, 128], fp8)
            nc.sync.dma_start(out=wt_sb, in_=wtf8_ext[:])
            wmt8_sb = consts.tile([128, NC8, N2], fp8)
            nc.sync.dma_start(out=wmt8_sb, in_=wmt8_ext[:])
            wb_sb = consts.tile([128, K2C], fp32)
            nc.sync.dma_start(out=wb_sb, in_=wb_ext[:])
            wmbr_sb = consts.tile([1, N2], bf16)
            nc.sync.dma_start(out=wmbr_sb, in_=wmbr_ext[:])
            whw64_sb = consts.tile([1, N2], bf16)
            nc.sync.dma_start(out=whw64_sb, in_=whw64_ext[:])
            # G: lhsT rows (ones@0, e_a@32, e_b@64) -> cols 64*e{a,b} - 64
            g65 = consts.tile([65, 2], bf16)
            nc.sync.dma_start(out=g65, in_=g65_ext[:])
            i97 = consts.tile([97, 4], bf16)
            nc.sync.dma_start(out=i97, in_=i97_ext[:])

            ones_row = consts.tile([1, 128], bf16)
            nc.vector.memset(ones_row, 1.0)
            ones1 = consts.tile([1, 1], bf16)
            nc.vector.memset(ones1, 1.0)
            ones8dr = consts.tile([128, 2, 16], fp8)
            nc.vector.memset(ones8dr, 1.0)

            m_sb = consts.tile([128, BLOC * NC8], fp32)
            c_sb = consts.tile([128, BLOC * NC8], fp32)
            # m/2 in fp8, nci pairs at [:, nci%2, 4b + nci//2]
            m8 = consts.tile([128, 2, 16], fp8)
            # batch b's cu row lives at partition 32b; other rows stay 0
            cu_rows = consts.tile([97, N], bf16)
            nc.vector.memset(cu_rows, 0.0)
            v8p = consts.tile([128, 2, 16], fp8)
            rs_col4 = consts.tile([128, BLOC], fp32)
            rs64_col4 = consts.tile([128, BLOC], fp32)
            sei4 = consts.tile([1, BLOC], fp32)
            # single exp-row tile: ones@0, e rows land on 32/64; the zero
            # rows in between never change, so zero it once outside the
            # loop (each batch's exp overwrites rows 32/64 after the
            # previous batch's transpose matmuls have read them).
            em = consts.tile([65, 2, TCH], bf16)
            nc.vector.memset(em, 0.0)
            nc.vector.memset(em[0:1, :, :], 1.0)
            if not cfg["end"]:
                nc.vector.memset(c_sb, 0.0)
            if not cfg["hm"]:
                nc.vector.memset(v8p, 0.01)

            xs = {}
            xtns = {}
            tails = {}

            def phase_a(b):
                """Load x[b] (two n-halves) + xf8[b] (four t-chunks);
                run each half's time-mean as soon as it lands.  For the
                later batches x goes first: its mean gates the end of the
                pipeline while xf8 has a full phase of slack."""
                xf8_sb = xp.tile([128, TC, NC8, TCH], fp8, tag="xf8",
                                 name=f"xf8_{b}", bufs=3)
                x_sb = xp.tile([128, NC8, T], bf16, tag="x",
                               name=f"x_{b}", bufs=2)

                def dma_xf8():
                    if cfg["xf8"]:
                        for t4 in range(TC):
                            nc.gpsimd.dma_start(out=xf8_sb[:, t4],
                                                in_=xf8_ext[b, t4])
                    elif cfg["hq"]:
                        nc.gpsimd.memset(xf8_sb[:, 0, 0:1, 0:16], 0.25)

                def dma_x():
                    if cfg["x"]:
                        for h in range(2):
                            nc.sync.dma_start(
                                out=x_sb[:, 4 * h:4 * h + 4, :],
                                in_=x_ext[b, h])
                    elif cfg["tree"]:
                        nc.gpsimd.memset(x_sb[:, 0:1, 0:16], 0.25)

                dma_xf8()
                dma_x()
                xs[b] = (x_sb, xf8_sb)
                if not cfg["tree"]:
                    return
                q = T // 4
                for h in range(2):
                    xh = x_sb[:, 4 * h:4 * h + 4, :]
                    mtr = scrp.tile([128, NC8 // 2, q], bf16, tag="mtr",
                                    name=f"mtr_{b}_{h}", bufs=2)
                    nc.vector.tensor_add(
                        mtr[:, :, :], xh[:, :, 0:q], xh[:, :, q:2 * q])
                    nc.vector.tensor_add(
                        mtr[:, :, :], mtr[:, :, :], xh[:, :, 2 * q:3 * q])
                    nc.vector.tensor_add(
                        mtr[:, :, :], mtr[:, :, :], xh[:, :, 3 * q:4 * q])
                    w = q
                    while w > 128:
                        nc.vector.tensor_add(
                            mtr[:, :, 0:w // 2], mtr[:, :, 0:w // 2],
                            mtr[:, :, w // 2:w])
                        w //= 2
                    nc.vector.reduce_sum(
                        out=m_sb[:, b * NC8 + 4 * h:b * NC8 + 4 * h + 4],
                        in_=mtr[:, :, 0:w], axis=AX.X)
                # m8[:, nci%2, 4b + nci//2] = m[nci] / 2
                nc.vector.tensor_scalar_mul(
                    m8[:, :, 4 * b:4 * b + 4],
                    m_sb[:, b * NC8:(b + 1) * NC8].rearrange(
                        "p (j k) -> p k j", k=2),
                    0.5)

            hms = {}

            def emit_hm_a(b):
                """hm row matmuls + tanh + v row (the ACT/DVE latency of
                this chain hides under the PE work emitted between _a and
                _b)."""
                hm_ps = sm_psp.tile([1, N2], fp32, tag="smps",
                                    name=f"hmps_{b}")
                nc.tensor.matmul(hm_ps, lhsT=ones1, rhs=wmbr_sb,
                                 start=True, stop=False)
                for j in range(NC8 // 2):
                    nc.tensor.matmul(
                        hm_ps, lhsT=m8[:, :, 4 * b + j:4 * b + j + 1],
                        rhs=wmt8_sb[:, 2 * j:2 * j + 2, :], start=False,
                        stop=(j == NC8 // 2 - 1), perf_mode=DR)
                hmt = smalls.tile([1, N2], bf16, tag="hmt", name=f"hmt_{b}")
                nc.scalar.activation(out=hmt, in_=hm_ps, func=AF.Tanh,
                                     scale=1.0 / (16.0 * T))
                vrow = smalls.tile([1, N2], bf16, tag="vrow",
                                   name=f"vrow_{b}")
                nc.vector.tensor_mul(vrow, hmt, whw64_sb)
                hms[b] = vrow

            def emit_hm_b(b):
                """transpose v row to fp8 columns."""
                vrow = hms[b]
                vps = sm_psp.tile([128, 2, 2], fp32, tag="smps",
                                  name=f"vps_{b}")
                for s in range(4):
                    nc.tensor.matmul(
                        vps[:, s % 2, s // 2:s // 2 + 1],
                        lhsT=vrow[0:1, ts(s, 128)], rhs=ones1,
                        start=True, stop=True)
                nc.vector.tensor_copy(out=v8p[:, :, 2 * b:2 * b + 2],
                                      in_=vps)

            def tail_alloc(b):
                dps = d_psp.tile([128, 2, 16], fp32, tag="dps",
                                 name=f"dps_{b}", bufs=1)
                cu_ps = d_psp.tile([1, 2 * TCH], fp32, tag="cups",
                                   name=f"cups_{b}", bufs=1)
                dcol = abp.tile([128, 2, 16], fp8, tag="dcol",
                                name=f"dcol_{b}", bufs=2)
                tails[b].update(dps=dps, cu_ps=cu_ps, dcol=dcol)

            def emit_sc(b, t4):
                tl = tails[b]
                hq8 = tl["hq8"]
                sc_ps = sm_psp.tile([1, TCH], fp32, tag="smps",
                                    name=f"scps_{b}_{t4}")
                for pj in range(2):
                    nc.tensor.matmul(
                        sc_ps,
                        lhsT=v8p[:, :, 2 * b + pj:2 * b + pj + 1],
                        rhs=hq8[:, 2 * pj:2 * pj + 2, ts(t4, TCH)],
                        start=(pj == 0), stop=(pj == 1), perf_mode=DR)
                half, r = t4 // 2, 32 * (t4 % 2 + 1)
                nc.scalar.activation(out=em[r:r + 1, half, :],
                                     in_=sc_ps, func=AF.Exp, scale=1.0 / 64.0)

            def emit_tr(b, half):
                # transpose exp rows to 64*(e-1) delta columns
                tl = tails[b]
                for s in range(4):
                    j = 4 * half + 2 * (s // 2)
                    nc.tensor.matmul(
                        tl["dps"][:, s % 2, j:j + 2],
                        lhsT=em[0:65, half, ts(s, 128)],
                        rhs=g65, start=True, stop=True)
                nc.scalar.activation(
                    out=tl["dcol"][:, :, 4 * half:4 * half + 4],
                    in_=tl["dps"][:, :, 4 * half:4 * half + 4],
                    func=AF.Copy)

            def emit_wsum(b, half):
                tl = tails[b]
                xtn_sb = tl["xtn"]
                for pj in range(4 * half, 4 * half + 4):
                    for h in range(2):
                        nc.tensor.matmul(
                            tl["cu_ps"][0:1, ts(h, TCH)],
                            lhsT=tl["dcol"][:, :, IDX[pj]:IDX[pj] + 1],
                            rhs=xtn_sb[:, pj, :, ts(h, TCH)],
                            start=(pj == 0), stop=(pj == T // 256 - 1),
                            perf_mode=DR)

            def emit_fin(b):
                tl = tails[b]
                # sum(e) = sum(dcol)/64 + T via tiny accumulating matmuls
                st_ps = tl["dps"][0:1, 1, 15:16]
                for pj in range(8):
                    nc.tensor.matmul(
                        st_ps, lhsT=tl["dcol"][:, :, IDX[pj]:IDX[pj] + 1],
                        rhs=ones8dr[:, :, 0:1], start=(pj == 0),
                        stop=(pj == 7), perf_mode=DR)
                nc.vector.tensor_copy(out=cu_rows[32 * b:32 * b + 1, :],
                                      in_=tl["cu_ps"])
                # sei = sum(dcol)/64 + T on ACT (keeps the DVE queue out of
                # the PE-blocking path); the rs chain runs in phase_end.
                nc.scalar.activation(out=sei4[0:1, b:b + 1], in_=st_ps,
                                     func=AF.Copy, scale=1.0 / 64.0,
                                     bias=float(T))

            def tail_piece(b, t4):
                """Emitted at the START of hq window t4 of the next batch:
                every dependency is at least one full window old."""
                if not cfg["sc"]:
                    return
                if t4 == 0:
                    tail_alloc(b)
                emit_sc(b, t4)
                if t4 == 2 and cfg["tr"]:
                    emit_tr(b, 0)
                elif t4 == 3 and cfg["wsum"]:
                    emit_wsum(b, 0)

            def tail_end(b):
                if not cfg["sc"]:
                    return
                if cfg["tr"]:
                    emit_tr(b, 1)
                if cfg["wsum"]:
                    emit_wsum(b, 1)
                if cfg["fin"]:
                    emit_fin(b)

            def phase_hqt(b):
                """hq matmuls + tanh for batch b, with batch b-1's tail
                pieces interleaved between t-chunk groups."""
                _, xf8_sb = xs[b]
                xtn_sb = xp.tile([128, T // 256, 2, N], fp8, tag="xtn",
                                 name=f"xtn_{b}", bufs=2)
                if cfg["xtn"]:
                    # halves by t-pair: the wsum for pj 0-3 starts as soon
                    # as the first half lands
                    nc.gpsimd.dma_start(out=xtn_sb[:, 0:4],
                                        in_=xtn_ext[b, :, 0:4])
                    nc.gpsimd.dma_start(out=xtn_sb[:, 4:8],
                                        in_=xtn_ext[b, :, 4:8])
                elif cfg["wsum"]:
                    nc.gpsimd.memset(xtn_sb[:, 0, 0:1, 0:16], 0.25)
                hq8 = hqp.tile([128, K2C, T], fp8, tag="hq",
                               name=f"hq_{b}", bufs=2)
                tails[b] = {"hq8": hq8, "xtn": xtn_sb}
                for t4 in range(TC if cfg["hq"] else 0):
                    if b >= 1:
                        tail_piece(b - 1, t4)
                    for k2c in range(K2C):
                        hq_ps = hq_psp.tile([128, TCH], fp32, tag="hqps",
                                            name=f"hqps_{b}_{t4}_{k2c}")
                        for np_ in range(NC8 // 2):
                            nc.tensor.matmul(
                                hq_ps,
                                lhsT=wt_sb[:, 2 * np_:2 * np_ + 2, k2c, :],
                                rhs=xf8_sb[:, t4, 2 * np_:2 * np_ + 2, :],
                                start=(np_ == 0), stop=(np_ == NC8 // 2 - 1),
                                perf_mode=DR)
                        if cfg["tanh"]:
                            nc.scalar.activation(
                                out=hq8[:, k2c, ts(t4, TCH)], in_=hq_ps,
                                func=AF.Tanh, scale=1.0 / 32.0,
                                bias=wb_sb[:, k2c:k2c + 1])
                if cfg["hm"]:
                    emit_hm_a(b)
                if b >= 1:
                    tail_end(b - 1)
                if cfg["hm"]:
                    emit_hm_b(b)

            def phase_end():
                for t4 in range(TC):
                    tail_piece(BLOC - 1, t4)
                tail_end(BLOC - 1)
                if not cfg["end"]:
                    nc.sync.dma_start(out=out_ext[:], in_=c_sb)
                    return
                # rs = 1/sum(e) for all four batches, broadcast to columns
                rs4 = smalls.tile([1, BLOC], fp32, tag="rs4", name="rs4")
                nc.vector.reciprocal(rs4, sei4)
                rs_bf4 = smalls.tile([1, BLOC], bf16, tag="rsbf4",
                                     name="rsbf4")
                nc.vector.tensor_copy(out=rs_bf4, in_=rs4)
                rc_ps = d_psp.tile([128, BLOC], fp32, tag="dps",
                                   name="rcps")
                nc.tensor.matmul(rc_ps, lhsT=ones_row, rhs=rs_bf4,
                                 start=True, stop=True)
                nc.vector.tensor_copy(out=rs_col4, in_=rc_ps)
                nc.scalar.activation(out=rs64_col4, in_=rc_ps,
                                     func=AF.Copy, scale=1.0 / 64.0)
                cuc_ps = sm_psp.tile([128, NC8, BLOC], fp32, tag="smps",
                                     name="cucps")
                for s in range(NC8):
                    nc.tensor.matmul(cuc_ps[:, s, :],
                                     lhsT=cu_rows[0:97, ts(s, 128)],
                                     rhs=i97, start=True, stop=True)
                for b in range(BLOC):
                    bc = slice(b * NC8, (b + 1) * NC8)
                    t1 = smalls.tile([128, NC8], fp32, tag="t1",
                                     name=f"t1_{b}")
                    nc.vector.tensor_scalar_mul(t1, cuc_ps[:, :, b],
                                                rs64_col4[:, b:b + 1])
                    t2 = smalls.tile([128, NC8], fp32, tag="t2",
                                     name=f"t2_{b}")
                    nc.vector.tensor_scalar_mul(t2, m_sb[:, bc],
                                                rs_col4[:, b:b + 1])
                    nc.vector.tensor_add(c_sb[:, bc], t1, t2)
                nc.sync.dma_start(out=out_ext[:], in_=c_sb)

            loop_ctx = (tc.For_i(0, loop_n, 1) if loop_n
                        else contextlib.nullcontext())
            with loop_ctx:
                phase_a(0)
                phase_a(1)
                phase_hqt(0)
                phase_a(2)
                phase_hqt(1)
                phase_a(3)
                phase_hqt(2)
                phase_hqt(3)
                phase_end()

    nc.compile()
    return nc


def _get_nc():
    if "nc" not in _CACHE:
        _CACHE["nc"] = _build()
    return _CACHE["nc"]


def benchmark(in_maps, iters=30, warmup=3, nc=None):
    """Time the compiled SPMD NEFF via repeated pipelined PJRT executions.

    The NTFF profile hook is unavailable in this container, so this is the
    closest proxy for HW exec time: inputs live on device, `iters` async
    dispatches are queued back-to-back, and we block once at the end.
    Returns estimated ns per execution.
    """
    import time

    import jax
    import numpy as np_
    from jax.sharding import Mesh, NamedSharding, PartitionSpec

    from concourse import mybir
    from concourse.bass2jax import (_bass_exec_p, install_neuronx_cc_hook,
                                    partition_id_tensor)

    install_neuronx_cc_hook()
    if nc is None:
        nc = _get_nc()

    partition_name = (nc.partition_id_tensor.name
                      if nc.partition_id_tensor else None)
    in_names, out_names, out_avals = [], [], []
    zero_outs = []
    for alloc in nc.m.functions[0].allocations:
        if not isinstance(alloc, mybir.MemoryLocationSet):
            continue
        name = alloc.memorylocations[0].name
        if alloc.kind == "ExternalInput":
            if name != partition_name:
                in_names.append(name)
        elif alloc.kind == "ExternalOutput":
            shape = tuple(alloc.tensor_shape)
            dtype = mybir.dt.np(alloc.dtype)
            out_names.append(name)
            out_avals.append(jax.core.ShapedArray(shape, dtype))
            zero_outs.append(np_.zeros(shape, dtype))
    n_params = len(in_names)
    all_in_names = in_names + out_names
    if partition_name is not None:
        all_in_names = all_in_names + [partition_name]

    def _body(*args):
        operands = list(args)
        if partition_name is not None:
            operands.append(partition_id_tensor())
        return tuple(_bass_exec_p.bind(
            *operands,
            out_avals=tuple(out_avals),
            in_names=tuple(all_in_names),
            out_names=tuple(out_names),
            lowering_input_output_aliases=(),
            sim_require_finite=True,
            sim_require_nnan=True,
            nc=nc,
        ))

    from jax.experimental.shard_map import shard_map

    devices = jax.devices()[:NCORES]
    mesh = Mesh(np_.asarray(devices), ("core",))
    spec = PartitionSpec("core")
    fn = jax.jit(shard_map(
        _body, mesh=mesh, in_specs=(spec,) * (n_params + len(out_names)),
        out_specs=(spec,) * len(out_names), check_rep=False))

    sharding = NamedSharding(mesh, spec)
    concat_in = [
        jax.device_put(
            np_.concatenate([np_.asarray(in_maps[c][nm]) for c in
                             range(NCORES)], axis=0), sharding)
        for nm in in_names
    ]
    concat_zeros = [
        jax.device_put(
            np_.zeros((NCORES * z.shape[0], *z.shape[1:]), z.dtype), sharding)
        for z in zero_outs
    ]
    args = concat_in + concat_zeros

    for _ in range(warmup):
        outs = fn(*args)
    jax.block_until_ready(outs)

    t0 = time.perf_counter()
    results = [fn(*args) for _ in range(iters)]
    jax.block_until_ready(results)
    t1 = time.perf_counter()
    return (t1 - t0) / iters * 1e9


def kernel(**inputs):
    global LAST_EXEC_NS
    # The NTFF profile hook (antenv.axon_hooks) is absent in some axon
    # containers and the traced branch of run_bass_kernel_spmd hard-fails
    # on its import; force the untraced PJRT path.
    os.environ["BASS_NEVER_TRACE"] = "1"
    from concourse.bass_utils import run_bass_kernel_spmd

    hyp = np.asarray(inputs["hyp"], dtype=np.float32)    # [T, B, N]
    W_w = np.asarray(inputs["W_w"], dtype=np.float32)    # [N2, N]
    W_b = np.asarray(inputs["W_b"], dtype=np.float32)    # [N2]
    Wm_w = np.asarray(inputs["Wm_w"], dtype=np.float32)  # [N2, N]
    Wm_b = np.asarray(inputs["Wm_b"], dtype=np.float32)  # [N2]
    Wh_w = np.asarray(inputs["Wh_w"], dtype=np.float32)  # [1, N2]
    # Wh_b is unused: softmax is shift-invariant.

    bf = ml_dtypes.bfloat16
    f8 = ml_dtypes.float8_e4m3fn

    def wlayout(w, dtype):  # [N2, N] -> [p, nci, k2c, j]
        return np.ascontiguousarray(
            w.reshape(K2C, 128, NC8, 128).transpose(3, 2, 0, 1)).astype(dtype)

    # x32 prescale keeps the uniform(-1/32, 1/32) weights out of the e4m3
    # subnormal range; the hq tanh activation divides it back out.
    wtf8 = wlayout(W_w * 32.0, f8)
    # [p, nci, k2] = 32*Wm[k2, nci*128+p]
    wmt8 = np.ascontiguousarray(
        (32.0 * Wm_w).reshape(N2, NC8, 128).transpose(2, 1, 0)).astype(f8)
    wb = np.ascontiguousarray(W_b.reshape(K2C, 128).T)
    wmbr = np.ascontiguousarray((16.0 * T * Wm_b).astype(bf).reshape(1, N2))
    whw64 = np.ascontiguousarray((64.0 * Wh_w).astype(bf).reshape(1, N2))
    g65 = np.zeros((65, 2), dtype=bf)
    g65[0, :] = -64.0
    g65[32, 0] = 64.0
    g65[64, 1] = 64.0
    i97 = np.zeros((97, 4), dtype=bf)
    for j in range(4):
        i97[32 * j, j] = 1.0

    hyp_bf = hyp.astype(bf)  # [T, B, N]
    in_maps = []
    for c in range(NCORES):
        xsb = hyp_bf[:, c * BLOC:(c + 1) * BLOC, :]       # [T, 4, N]
        xsb = xsb.transpose(1, 2, 0)                      # [4, N, T]
        # n = nci*128 + p  ->  [b, p, nci, t]
        xsb = np.ascontiguousarray(
            xsb.reshape(BLOC, NC8, 128, T).transpose(0, 2, 1, 3))
        # x: [b, h, p, nci%4, t]
        xh = np.ascontiguousarray(
            xsb.reshape(BLOC, 128, 2, NC8 // 2, T).transpose(0, 2, 1, 3, 4))
        # xf8: [b, t4, p, nci, tch]
        xf8 = np.ascontiguousarray(
            xsb.reshape(BLOC, 128, NC8, TC, TCH)
            .transpose(0, 3, 1, 2, 4)).astype(f8)
        xtn = hyp_bf[:, c * BLOC:(c + 1) * BLOC, :]       # [T, 4, N]
        xtn = xtn.transpose(1, 0, 2)                      # [4, T, N]
        # t = tp*256 + k*128 + p  ->  [b, p, tp, k, n]
        xtn = np.ascontiguousarray(
            xtn.reshape(BLOC, T // 256, 2, 128, N)
            .transpose(0, 3, 1, 2, 4)).astype(f8)
        in_maps.append({
            "x": xh, "xf8": xf8, "xtn": xtn,
            "wtf8": wtf8, "wmt8": wmt8, "wb": wb, "wmbr": wmbr,
            "whw64": whw64, "g65": g65, "i97": i97,
        })

    nc = _get_nc()
    res = run_bass_kernel_spmd(nc, in_maps, list(range(NCORES)))
    LAST_EXEC_NS = res.exec_time_ns
    _CACHE["last_in_maps"] = in_maps

    parts = []
    for c in range(NCORES):
        r = np.asarray(res.results[c]["out"])             # [128, BLOC*NC8]
        r = r.reshape(128, BLOC, NC8).transpose(1, 2, 0).reshape(BLOC, N)
        parts.append(r)
    return np.ascontiguousarray(
        np.concatenate(parts, axis=0)).astype(np.float32)


# revision 63
# speedup vs baseline: 1.0062x; 1.0062x over previous
"""Trainium2 Bass kernel for nn_Attention_4741643894804 (sparse_attention).

Reference computation (T=2048, B=32, N=1024, N2=512):
    m  = mean_t hyp[t, b, :]                       # [B, N]
    x  = hyp.transpose(1, 0, 2)                    # [B, T, N]
    hq = tanh(x @ W_w.T + W_b)                     # [B, T, N2]
    hm = tanh(m @ Wm_w.T + Wm_b)                   # [B, N2]
    s  = (hq * hm[:, None, :]) @ Wh_w.T + Wh_b     # [B, T, 1]
    a  = softmax(s, axis=1)
    c  = sum_t a * x                               # [B, N]

Strategy: pure data parallel over B (4 batches per core, 8 cores, no
collectives).  Per batch:
  - hq^T = tanh(W_w @ x + W_b) as [k2, t] tiles on TensorE in fp8
    DoubleRow (e4m3; W_w prescaled x32 on the host to dodge subnormals,
    undone by the tanh activation's scale); tanh output stored fp8.
  - mean m_raw = sum_t x from bf16 x via in-place pairwise add trees on
    DVE (split per n-half so each half starts as soon as its DMA lands).
  - hm row s64[1, k2] via fp8 DoubleRow with the tiny mean as lhsT (m/2
    in fp8, Wm prescaled x32; no weight-load cost), wm_b folded in via a
    ones matmul against a host-prescaled bias row; v = 64*tanh(.)*Wh_w
    on ACT+DVE, transposed to fp8 columns with 4 tiny K=1 matmuls.
  - score rows s64[1, t] = v8 . hq8 via fp8 DoubleRow (x64 undone by the
    exp scale); e = exp(score) unnormalized (softmax is shift-invariant
    and scores are tiny for this data).
  - exp rows land on partitions 32/64 of a zeroed [65, 2, TCH] tile
    (ones row at 0); one K=65 matmul per 128-slice against a constant
    [[-64,-64],[64,0],[0,64]]-padded rhs transposes AND applies 64*(e-1)
    in one shot -> delta columns.
  - weighted sum c_u = sum_t 64*(e_t-1) x_t on TensorE against a [t, n]
    fp8 copy of x (DoubleRow); sum(e) = sum(dcol)/64 + T via 8 tiny
    accumulating matmuls, so no f32 reductions are needed.
  - c = rs*(cu/64 + m_raw), rs = 1/sum(e): the delta term is ~1% of c so
    it tolerates fp8; m_raw supplies the dominant part at bf16 precision.

Scheduling: batch b's score/softmax/weighted-sum tail is emitted
interleaved between batch b+1's hq matmul groups, so the in-order PE
never stalls on the ACT exp chain; hm(b) is emitted at the end of
HQT(b), a full batch ahead of its first use.
"""

import os
import sys

import numpy as np

if "/opt/trn_rl_repo" not in sys.path:
    sys.path.insert(0, "/opt/trn_rl_repo")

import ml_dtypes

T, B, N, N2 = 2048, 32, 1024, 512
NCORES = 8
BLOC = B // NCORES  # 4 batches per core
NC8 = N // 128      # 8 n-chunks of 128
K2C = N2 // 128     # 4 k2-chunks of 128
TCH = 512           # t tile (one PSUM bank of f32)
TC = T // TCH       # 4 t-chunks

# wsum matmul pj -> dcol column index (transposes write 2 adjacent cols)
IDX = [0, 2, 1, 3, 4, 6, 5, 7]

_CACHE = {}
LAST_EXEC_NS = None


def _build(loop_n=None, stage=3):
    """Build the Bass graph.  loop_n wraps the whole computation in an
    on-device For_i loop (benchmarking only).  stage ablates work for HW
    attribution: 0=DMA only, 1=+hq matmuls, 2=+tanh/score/exp/wsum,
    3=full (default)."""
    import contextlib

    import concourse.bass as bass
    import concourse.mybir as mybir
    import concourse.tile as tile
    from concourse import bacc
    from concourse.bass import ts

    fp32 = mybir.dt.float32
    bf16 = mybir.dt.bfloat16
    fp8 = mybir.dt.float8e4
    AF = mybir.ActivationFunctionType
    ALU = mybir.AluOpType
    AX = mybir.AxisListType
    DR = mybir.MatmulPerfMode.DoubleRow

    # ablation flags per stage
    ALL = dict(x=1, xf8=1, xtn=1, hq=1, tanh=1, sc=1, tr=1, wsum=1, fin=1,
               hm=1, tree=1, end=1)
    CFG = {
        0: dict(ALL, hq=0, tanh=0, sc=0, tr=0, wsum=0, fin=0, hm=0, tree=0,
                end=0),
        1: dict(ALL, tanh=0, sc=0, tr=0, wsum=0, fin=0, hm=0, tree=0, end=0),
        2: dict(ALL, hm=0, tree=0, end=0),
        3: ALL,
        4: dict(ALL, sc=0, tr=0, wsum=0, fin=0, hm=0, tree=0, end=0),
        5: dict(ALL, tr=0, wsum=0, fin=0, hm=0, tree=0, end=0),
        6: dict(ALL, wsum=0, fin=0, hm=0, tree=0, end=0),
        7: dict(ALL, fin=0, hm=0, tree=0, end=0),
        11: dict(ALL, x=0, xtn=0, tanh=0, sc=0, tr=0, wsum=0, fin=0, hm=0,
                 tree=0, end=0),
        12: dict(ALL, x=0, xtn=0, sc=0, tr=0, wsum=0, fin=0, hm=0, tree=0,
                 end=0),
        20: dict(ALL, x=0, xf8=0, xtn=0),
        21: dict(ALL, xf8=0, xtn=0),
    }
    cfg = CFG[stage]

    nc = bacc.Bacc("TRN2", target_bir_lowering=False, debug=False,
                   num_devices=NCORES)

    x_ext = nc.declare_dram_parameter("x", [BLOC, 2, 128, NC8 // 2, T], bf16,
                                      isOutput=False)
    xf8_ext = nc.declare_dram_parameter("xf8", [BLOC, TC, 128, NC8, TCH],
                                        fp8, isOutput=False)
    xtn_ext = nc.declare_dram_parameter(
        "xtn", [BLOC, 128, T // 256, 2, N], fp8, isOutput=False)
    wtf8_ext = nc.declare_dram_parameter("wtf8", [128, NC8, K2C, 128], fp8,
                                         isOutput=False)
    wmt8_ext = nc.declare_dram_parameter("wmt8", [128, NC8, N2], fp8,
                                         isOutput=False)
    wb_ext = nc.declare_dram_parameter("wb", [128, K2C], fp32, isOutput=False)
    wmbr_ext = nc.declare_dram_parameter("wmbr", [1, N2], bf16,
                                         isOutput=False)
    whw64_ext = nc.declare_dram_parameter("whw64", [1, N2], bf16,
                                          isOutput=False)
    g65_ext = nc.declare_dram_parameter("g65", [65, 2], bf16, isOutput=False)
    i97_ext = nc.declare_dram_parameter("i97", [97, 4], bf16, isOutput=False)
    out_ext = nc.declare_dram_parameter("out", [128, BLOC * NC8], fp32,
                                        isOutput=True)

    with tile.TileContext(nc) as tc:
        with (
            tc.tile_pool(name="consts", bufs=1) as consts,
            tc.tile_pool(name="xp", bufs=3) as xp,
            tc.tile_pool(name="hqp", bufs=2) as hqp,
            tc.tile_pool(name="scr", bufs=2) as scrp,
            tc.tile_pool(name="abp", bufs=2) as abp,
            tc.tile_pool(name="smalls", bufs=2) as smalls,
            tc.tile_pool(name="hq_ps", bufs=4, space="PSUM") as hq_psp,
            tc.tile_pool(name="sm_ps", bufs=1, space="PSUM") as sm_psp,
            tc.tile_pool(name="d_ps", bufs=1, space="PSUM") as d_psp,
        ):
            # wtf8 first: it gates the very first hq matmul
            wt_sb = consts.tile([128, NC8, K2C, 128], fp8)
            nc.sync.dma_start(out=wt_sb, in_=wtf8_ext[:])
            wmt8_sb = consts.tile([128, NC8, N2], fp8)
            nc.sync.dma_start(out=wmt8_sb, in_=wmt8_ext[:])
            wb_sb = consts.tile([128, K2C], fp32)
            nc.sync.dma_start(out=wb_sb, in_=wb_ext[:])
            wmbr_sb = consts.tile([1, N2], bf16)
            nc.sync.dma_start(out=wmbr_sb, in_=wmbr_ext[:])
            whw64_sb = consts.tile([1, N2], bf16)
            nc.sync.dma_start(out=whw64_sb, in_=whw64_ext[:])
            # G: lhsT rows (ones@0, e_a@32, e_b@64) -> cols 64*e{a,b} - 64
            g65 = consts.tile([65, 2], bf16)
            nc.sync.dma_start(out=g65, in_=g65_ext[:])
            i97 = consts.tile([97, 4], bf16)
            nc.sync.dma_start(out=i97, in_=i97_ext[:])

            ones_row = consts.tile([1, 128], bf16)
            nc.vector.memset(ones_row, 1.0)
            ones1 = consts.tile([1, 1], bf16)
            nc.vector.memset(ones1, 1.0)
            ones8dr = consts.tile([128, 2, 16], fp8)
            nc.vector.memset(ones8dr, 1.0)

            m_sb = consts.tile([128, BLOC * NC8], fp32)
            c_sb = consts.tile([128, BLOC * NC8], fp32)
            # m/2 in fp8, nci pairs at [:, nci%2, 4b + nci//2]
            m8 = consts.tile([128, 2, 16], fp8)
            # batch b's cu row lives at partition 32b; other rows stay 0
            cu_rows = consts.tile([97, N], bf16)
            nc.vector.memset(cu_rows, 0.0)
            v8p = consts.tile([128, 2, 16], fp8)
            rs_col4 = consts.tile([128, BLOC], fp32)
            rs64_col4 = consts.tile([128, BLOC], fp32)
            sei4 = consts.tile([1, BLOC], fp32)
            # single exp-row tile: ones@0, e rows land on 32/64; the zero
            # rows in between never change, so zero it once outside the
            # loop (each batch's exp overwrites rows 32/64 after the
            # previous batch's transpose matmuls have read them).
            em = consts.tile([65, 2, TCH], bf16)
            nc.vector.memset(em, 0.0)
            nc.vector.memset(em[0:1, :, :], 1.0)
            if not cfg["end"]:
                nc.vector.memset(c_sb, 0.0)
            if not cfg["hm"]:
                nc.vector.memset(v8p, 0.01)

            xs = {}
            xtns = {}
            tails = {}

            def phase_a(b):
                """Load x[b] (two n-halves) + xf8[b] (four t-chunks);
                run each half's time-mean as soon as it lands.  For the
                later batches x goes first: its mean gates the end of the
                pipeline while xf8 has a full phase of slack."""
                xf8_sb = xp.tile([128, TC, NC8, TCH], fp8, tag="xf8",
                                 name=f"xf8_{b}", bufs=3)
                x_sb = xp.tile([128, NC8, T], bf16, tag="x",
                               name=f"x_{b}", bufs=2)

                def dma_xf8():
                    if cfg["xf8"]:
                        for t4 in range(TC):
                            nc.gpsimd.dma_start(out=xf8_sb[:, t4],
                                                in_=xf8_ext[b, t4])
                    elif cfg["hq"]:
                        nc.gpsimd.memset(xf8_sb[:, 0, 0:1, 0:16], 0.25)

                def dma_x():
                    if cfg["x"]:
                        for h in range(2):
                            nc.sync.dma_start(
                                out=x_sb[:, 4 * h:4 * h + 4, :],
                                in_=x_ext[b, h])
                    elif cfg["tree"]:
                        nc.gpsimd.memset(x_sb[:, 0:1, 0:16], 0.25)

                dma_xf8()
                dma_x()
                xs[b] = (x_sb, xf8_sb)
                if not cfg["tree"]:
                    return
                q = T // 4
                for h in range(2):
                    xh = x_sb[:, 4 * h:4 * h + 4, :]
                    mtr = scrp.tile([128, NC8 // 2, q], bf16, tag="mtr",
                                    name=f"mtr_{b}_{h}", bufs=2)
                    nc.vector.tensor_add(
                        mtr[:, :, :], xh[:, :, 0:q], xh[:, :, q:2 * q])
                    nc.vector.tensor_add(
                        mtr[:, :, :], mtr[:, :, :], xh[:, :, 2 * q:3 * q])
                    nc.vector.tensor_add(
                        mtr[:, :, :], mtr[:, :, :], xh[:, :, 3 * q:4 * q])
                    w = q
                    while w > 128:
                        nc.vector.tensor_add(
                            mtr[:, :, 0:w // 2], mtr[:, :, 0:w // 2],
                            mtr[:, :, w // 2:w])
                        w //= 2
                    nc.vector.reduce_sum(
                        out=m_sb[:, b * NC8 + 4 * h:b * NC8 + 4 * h + 4],
                        in_=mtr[:, :, 0:w], axis=AX.X)
                # m8[:, nci%2, 4b + nci//2] = m[nci] / 2
                nc.vector.tensor_scalar_mul(
                    m8[:, :, 4 * b:4 * b + 4],
                    m_sb[:, b * NC8:(b + 1) * NC8].rearrange(
                        "p (j k) -> p k j", k=2),
                    0.5)

            hms = {}

            def emit_hm_a(b):
                """hm row matmuls + tanh + v row (the ACT/DVE latency of
                this chain hides under the PE work emitted between _a and
                _b)."""
                hm_ps = sm_psp.tile([1, N2], fp32, tag="smps",
                                    name=f"hmps_{b}")
                nc.tensor.matmul(hm_ps, lhsT=ones1, rhs=wmbr_sb,
                                 start=True, stop=False)
                for j in range(NC8 // 2):
                    nc.tensor.matmul(
                        hm_ps, lhsT=m8[:, :, 4 * b + j:4 * b + j + 1],
                        rhs=wmt8_sb[:, 2 * j:2 * j + 2, :], start=False,
                        stop=(j == NC8 // 2 - 1), perf_mode=DR)
                hmt = smalls.tile([1, N2], bf16, tag="hmt", name=f"hmt_{b}")
                nc.scalar.activation(out=hmt, in_=hm_ps, func=AF.Tanh,
                                     scale=1.0 / (16.0 * T))
                vrow = smalls.tile([1, N2], bf16, tag="vrow",
                                   name=f"vrow_{b}")
                nc.vector.tensor_mul(vrow, hmt, whw64_sb)
                hms[b] = vrow

            def emit_hm_b(b):
                """transpose v row to fp8 columns."""
                vrow = hms[b]
                vps = sm_psp.tile([128, 2, 2], fp32, tag="smps",
                                  name=f"vps_{b}")
                for s in range(4):
                    nc.tensor.matmul(
                        vps[:, s % 2, s // 2:s // 2 + 1],
                        lhsT=vrow[0:1, ts(s, 128)], rhs=ones1,
                        start=True, stop=True)
                nc.vector.tensor_copy(out=v8p[:, :, 2 * b:2 * b + 2],
                                      in_=vps)

            def tail_alloc(b):
                dps = d_psp.tile([128, 2, 16], fp32, tag="dps",
                                 name=f"dps_{b}", bufs=1)
                cu_ps = d_psp.tile([1, 2 * TCH], fp32, tag="cups",
                                   name=f"cups_{b}", bufs=1)
                dcol = abp.tile([128, 2, 16], fp8, tag="dcol",
                                name=f"dcol_{b}", bufs=2)
                tails[b].update(dps=dps, cu_ps=cu_ps, dcol=dcol)

            def emit_sc(b, t4):
                tl = tails[b]
                hq8 = tl["hq8"]
                sc_ps = sm_psp.tile([1, TCH], fp32, tag="smps",
                                    name=f"scps_{b}_{t4}")
                for pj in range(2):
                    nc.tensor.matmul(
                        sc_ps,
                        lhsT=v8p[:, :, 2 * b + pj:2 * b + pj + 1],
                        rhs=hq8[:, 2 * pj:2 * pj + 2, ts(t4, TCH)],
                        start=(pj == 0), stop=(pj == 1), perf_mode=DR)
                half, r = t4 // 2, 32 * (t4 % 2 + 1)
                nc.scalar.activation(out=em[r:r + 1, half, :],
                                     in_=sc_ps, func=AF.Exp, scale=1.0 / 64.0)

            def emit_tr(b, half):
                # transpose exp rows to 64*(e-1) delta columns
                tl = tails[b]
                for s in range(4):
                    j = 4 * half + 2 * (s // 2)
                    nc.tensor.matmul(
                        tl["dps"][:, s % 2, j:j + 2],
                        lhsT=em[0:65, half, ts(s, 128)],
                        rhs=g65, start=True, stop=True)
                nc.scalar.activation(
                    out=tl["dcol"][:, :, 4 * half:4 * half + 4],
                    in_=tl["dps"][:, :, 4 * half:4 * half + 4],
                    func=AF.Copy)

            def emit_wsum(b, half):
                tl = tails[b]
                xtn_sb = tl["xtn"]
                for pj in range(4 * half, 4 * half + 4):
                    for h in range(2):
                        nc.tensor.matmul(
                            tl["cu_ps"][0:1, ts(h, TCH)],
                            lhsT=tl["dcol"][:, :, IDX[pj]:IDX[pj] + 1],
                            rhs=xtn_sb[:, pj, :, ts(h, TCH)],
                            start=(pj == 0), stop=(pj == T // 256 - 1),
                            perf_mode=DR)

            def emit_fin(b):
                tl = tails[b]
                # sum(e) = sum(dcol)/64 + T via tiny accumulating matmuls
                st_ps = tl["dps"][0:1, 1, 15:16]
                for pj in range(8):
                    nc.tensor.matmul(
                        st_ps, lhsT=tl["dcol"][:, :, IDX[pj]:IDX[pj] + 1],
                        rhs=ones8dr[:, :, 0:1], start=(pj == 0),
                        stop=(pj == 7), perf_mode=DR)
                nc.vector.tensor_copy(out=cu_rows[32 * b:32 * b + 1, :],
                                      in_=tl["cu_ps"])
                # sei = sum(dcol)/64 + T on ACT (keeps the DVE queue out of
                # the PE-blocking path); the rs chain runs in phase_end.
                nc.scalar.activation(out=sei4[0:1, b:b + 1], in_=st_ps,
                                     func=AF.Copy, scale=1.0 / 64.0,
                                     bias=float(T))

            def tail_piece(b, t4):
                """Emitted at the START of hq window t4 of the next batch:
                every dependency is at least one full window old."""
                if not cfg["sc"]:
                    return
                if t4 == 0:
                    tail_alloc(b)
                emit_sc(b, t4)
                if t4 == 2 and cfg["tr"]:
                    emit_tr(b, 0)
                elif t4 == 3 and cfg["wsum"]:
                    emit_wsum(b, 0)

            def tail_end(b):
                if not cfg["sc"]:
                    return
                if cfg["tr"]:
                    emit_tr(b, 1)
                if cfg["wsum"]:
                    emit_wsum(b, 1)
                if cfg["fin"]:
                    emit_fin(b)

            def phase_hqt(b):
                """hq matmuls + tanh for batch b, with batch b-1's tail
                pieces interleaved between t-chunk groups."""
                _, xf8_sb = xs[b]
                xtn_sb = xp.tile([128, T // 256, 2, N], fp8, tag="xtn",
                                 name=f"xtn_{b}", bufs=2)
                if cfg["xtn"]:
                    nc.gpsimd.dma_start(out=xtn_sb, in_=xtn_ext[b])
                elif cfg["wsum"]:
                    nc.gpsimd.memset(xtn_sb[:, 0, 0:1, 0:16], 0.25)
                hq8 = hqp.tile([128, K2C, T], fp8, tag="hq",
                               name=f"hq_{b}", bufs=2)
                tails[b] = {"hq8": hq8, "xtn": xtn_sb}
                for t4 in range(TC if cfg["hq"] else 0):
                    if b >= 1:
                        tail_piece(b - 1, t4)
                    for k2c in range(K2C):
                        hq_ps = hq_psp.tile([128, TCH], fp32, tag="hqps",
                                            name=f"hqps_{b}_{t4}_{k2c}")
                        for np_ in range(NC8 // 2):
                            nc.tensor.matmul(
                                hq_ps,
                                lhsT=wt_sb[:, 2 * np_:2 * np_ + 2, k2c, :],
                                rhs=xf8_sb[:, t4, 2 * np_:2 * np_ + 2, :],
                                start=(np_ == 0), stop=(np_ == NC8 // 2 - 1),
                                perf_mode=DR)
                        if cfg["tanh"]:
                            nc.scalar.activation(
                                out=hq8[:, k2c, ts(t4, TCH)], in_=hq_ps,
                                func=AF.Tanh, scale=1.0 / 32.0,
                                bias=wb_sb[:, k2c:k2c + 1])
                if cfg["hm"]:
                    emit_hm_a(b)
                if b >= 1:
                    tail_end(b - 1)
                if cfg["hm"]:
                    emit_hm_b(b)

            def phase_end():
                for t4 in range(TC):
                    tail_piece(BLOC - 1, t4)
                tail_end(BLOC - 1)
                if not cfg["end"]:
                    nc.sync.dma_start(out=out_ext[:], in_=c_sb)
                    return
                # rs = 1/sum(e) for all four batches, broadcast to columns
                rs4 = smalls.tile([1, BLOC], fp32, tag="rs4", name="rs4")
                nc.vector.reciprocal(rs4, sei4)
                rs_bf4 = smalls.tile([1, BLOC], bf16, tag="rsbf4",
                                     name="rsbf4")
                nc.vector.tensor_copy(out=rs_bf4, in_=rs4)
                rc_ps = d_psp.tile([128, BLOC], fp32, tag="dps",
                                   name="rcps")
                nc.tensor.matmul(rc_ps, lhsT=ones_row, rhs=rs_bf4,
                                 start=True, stop=True)
                nc.vector.tensor_copy(out=rs_col4, in_=rc_ps)
                nc.scalar.activation(out=rs64_col4, in_=rc_ps,
                                     func=AF.Copy, scale=1.0 / 64.0)
                cuc_ps = sm_psp.tile([128, NC8, BLOC], fp32, tag="smps",
                                     name="cucps")
                for s in range(NC8):
                    nc.tensor.matmul(cuc_ps[:, s, :],
                                     lhsT=cu_rows[0:97, ts(s, 128)],
                                     rhs=i97, start=True, stop=True)
                for b in range(BLOC):
                    bc = slice(b * NC8, (b + 1) * NC8)
                    t1 = smalls.tile([128, NC8], fp32, tag="t1",
                                     name=f"t1_{b}")
                    nc.vector.tensor_scalar_mul(t1, cuc_ps[:, :, b],
                                                rs64_col4[:, b:b + 1])
                    t2 = smalls.tile([128, NC8], fp32, tag="t2",
                                     name=f"t2_{b}")
                    nc.vector.tensor_scalar_mul(t2, m_sb[:, bc],
                                                rs_col4[:, b:b + 1])
                    nc.vector.tensor_add(c_sb[:, bc], t1, t2)
                nc.sync.dma_start(out=out_ext[:], in_=c_sb)

            loop_ctx = (tc.For_i(0, loop_n, 1) if loop_n
                        else contextlib.nullcontext())
            with loop_ctx:
                phase_a(0)
                phase_a(1)
                phase_hqt(0)
                phase_a(2)
                phase_hqt(1)
                phase_a(3)
                phase_hqt(2)
                phase_hqt(3)
                phase_end()

    nc.compile()
    return nc


def _get_nc():
    if "nc" not in _CACHE:
        _CACHE["nc"] = _build()
    return _CACHE["nc"]


def benchmark(in_maps, iters=30, warmup=3, nc=None):
    """Time the compiled SPMD NEFF via repeated pipelined PJRT executions.

    The NTFF profile hook is unavailable in this container, so this is the
    closest proxy for HW exec time: inputs live on device, `iters` async
    dispatches are queued back-to-back, and we block once at the end.
    Returns estimated ns per execution.
    """
    import time

    import jax
    import numpy as np_
    from jax.sharding import Mesh, NamedSharding, PartitionSpec

    from concourse import mybir
    from concourse.bass2jax import (_bass_exec_p, install_neuronx_cc_hook,
                                    partition_id_tensor)

    install_neuronx_cc_hook()
    if nc is None:
        nc = _get_nc()

    partition_name = (nc.partition_id_tensor.name
                      if nc.partition_id_tensor else None)
    in_names, out_names, out_avals = [], [], []
    zero_outs = []
    for alloc in nc.m.functions[0].allocations:
        if not isinstance(alloc, mybir.MemoryLocationSet):
            continue
        name = alloc.memorylocations[0].name
        if alloc.kind == "ExternalInput":
            if name != partition_name:
                in_names.append(name)
        elif alloc.kind == "ExternalOutput":
            shape = tuple(alloc.tensor_shape)
            dtype = mybir.dt.np(alloc.dtype)
            out_names.append(name)
            out_avals.append(jax.core.ShapedArray(shape, dtype))
            zero_outs.append(np_.zeros(shape, dtype))
    n_params = len(in_names)
    all_in_names = in_names + out_names
    if partition_name is not None:
        all_in_names = all_in_names + [partition_name]

    def _body(*args):
        operands = list(args)
        if partition_name is not None:
            operands.append(partition_id_tensor())
        return tuple(_bass_exec_p.bind(
            *operands,
            out_avals=tuple(out_avals),
            in_names=tuple(all_in_names),
            out_names=tuple(out_names),
            lowering_input_output_aliases=(),
            sim_require_finite=True,
            sim_require_nnan=True,
            nc=nc,
        ))

    from jax.experimental.shard_map import shard_map

    devices = jax.devices()[:NCORES]
    mesh = Mesh(np_.asarray(devices), ("core",))
    spec = PartitionSpec("core")
    fn = jax.jit(shard_map(
        _body, mesh=mesh, in_specs=(spec,) * (n_params + len(out_names)),
        out_specs=(spec,) * len(out_names), check_rep=False))

    sharding = NamedSharding(mesh, spec)
    concat_in = [
        jax.device_put(
            np_.concatenate([np_.asarray(in_maps[c][nm]) for c in
                             range(NCORES)], axis=0), sharding)
        for nm in in_names
    ]
    concat_zeros = [
        jax.device_put(
            np_.zeros((NCORES * z.shape[0], *z.shape[1:]), z.dtype), sharding)
        for z in zero_outs
    ]
    args = concat_in + concat_zeros

    for _ in range(warmup):
        outs = fn(*args)
    jax.block_until_ready(outs)

    t0 = time.perf_counter()
    results = [fn(*args) for _ in range(iters)]
    jax.block_until_ready(results)
    t1 = time.perf_counter()
    return (t1 - t0) / iters * 1e9


def kernel(**inputs):
    global LAST_EXEC_NS
    # The NTFF profile hook (antenv.axon_hooks) is absent in some axon
    # containers and the traced branch of run_bass_kernel_spmd hard-fails
    # on its import; force the untraced PJRT path.
    os.environ["BASS_NEVER_TRACE"] = "1"
    from concourse.bass_utils import run_bass_kernel_spmd

    hyp = np.asarray(inputs["hyp"], dtype=np.float32)    # [T, B, N]
    W_w = np.asarray(inputs["W_w"], dtype=np.float32)    # [N2, N]
    W_b = np.asarray(inputs["W_b"], dtype=np.float32)    # [N2]
    Wm_w = np.asarray(inputs["Wm_w"], dtype=np.float32)  # [N2, N]
    Wm_b = np.asarray(inputs["Wm_b"], dtype=np.float32)  # [N2]
    Wh_w = np.asarray(inputs["Wh_w"], dtype=np.float32)  # [1, N2]
    # Wh_b is unused: softmax is shift-invariant.

    bf = ml_dtypes.bfloat16
    f8 = ml_dtypes.float8_e4m3fn

    def wlayout(w, dtype):  # [N2, N] -> [p, nci, k2c, j]
        return np.ascontiguousarray(
            w.reshape(K2C, 128, NC8, 128).transpose(3, 2, 0, 1)).astype(dtype)

    # x32 prescale keeps the uniform(-1/32, 1/32) weights out of the e4m3
    # subnormal range; the hq tanh activation divides it back out.
    wtf8 = wlayout(W_w * 32.0, f8)
    # [p, nci, k2] = 32*Wm[k2, nci*128+p]
    wmt8 = np.ascontiguousarray(
        (32.0 * Wm_w).reshape(N2, NC8, 128).transpose(2, 1, 0)).astype(f8)
    wb = np.ascontiguousarray(W_b.reshape(K2C, 128).T)
    wmbr = np.ascontiguousarray((16.0 * T * Wm_b).astype(bf).reshape(1, N2))
    whw64 = np.ascontiguousarray((64.0 * Wh_w).astype(bf).reshape(1, N2))
    g65 = np.zeros((65, 2), dtype=bf)
    g65[0, :] = -64.0
    g65[32, 0] = 64.0
    g65[64, 1] = 64.0
    i97 = np.zeros((97, 4), dtype=bf)
    for j in range(4):
        i97[32 * j, j] = 1.0

    hyp_bf = hyp.astype(bf)  # [T, B, N]
    in_maps = []
    for c in range(NCORES):
        xsb = hyp_bf[:, c * BLOC:(c + 1) * BLOC, :]       # [T, 4, N]
        xsb = xsb.transpose(1, 2, 0)                      # [4, N, T]
        # n = nci*128 + p  ->  [b, p, nci, t]
        xsb = np.ascontiguousarray(
            xsb.reshape(BLOC, NC8, 128, T).transpose(0, 2, 1, 3))
        # x: [b, h, p, nci%4, t]
        xh = np.ascontiguousarray(
            xsb.reshape(BLOC, 128, 2, NC8 // 2, T).transpose(0, 2, 1, 3, 4))
        # xf8: [b, t4, p, nci, tch]
        xf8 = np.ascontiguousarray(
            xsb.reshape(BLOC, 128, NC8, TC, TCH)
            .transpose(0, 3, 1, 2, 4)).astype(f8)
        xtn = hyp_bf[:, c * BLOC:(c + 1) * BLOC, :]       # [T, 4, N]
        xtn = xtn.transpose(1, 0, 2)                      # [4, T, N]
        # t = tp*256 + k*128 + p  ->  [b, p, tp, k, n]
        xtn = np.ascontiguousarray(
            xtn.reshape(BLOC, T // 256, 2, 128, N)
            .transpose(0, 3, 1, 2, 4)).astype(f8)
        in_maps.append({
            "x": xh, "xf8": xf8, "xtn": xtn,
            "wtf8": wtf8, "wmt8": wmt8, "wb": wb, "wmbr": wmbr,
            "whw64": whw64, "g65": g65, "i97": i97,
        })

    nc = _get_nc()
    res = run_bass_kernel_spmd(nc, in_maps, list(range(NCORES)))
    LAST_EXEC_NS = res.exec_time_ns
    _CACHE["last_in_maps"] = in_maps

    parts = []
    for c in range(NCORES):
        r = np.asarray(res.results[c]["out"])             # [128, BLOC*NC8]
        r = r.reshape(128, BLOC, NC8).transpose(1, 2, 0).reshape(BLOC, N)
        parts.append(r)
    return np.ascontiguousarray(
        np.concatenate(parts, axis=0)).astype(np.float32)
